# revision 29
# baseline (speedup 1.0000x reference)
"""Trainium2 Bass kernel for EnhancedMessageLayer (GNN message passing).

Strategy (8 NeuronCores, no collectives):
  * Nodes split into 8 contiguous slices of 6250; every edge is owned by the
    core that owns its dst node.  Each core computes the full layer for its
    node slice; host concatenates.  Per core, nodes are processed in 49 tiles
    of 128 (last tile overlaps; duplicated rows discarded on host).
  * Host prep lays out, per (core, tile), the transposed bf16 panels
    x[src].T and x[dst].T ([h, e] slot layout, matmul-stationary-ready),
    edge_attr blocks (with a ones-row so b_msg1 rides the same matmul), the
    per-slot dst-slot columns, and per-node degree.  No device-side gather,
    no dense one-hot uploads.
  * Device, pass 1+2 (per tile): per 128-edge chunk, layer 1 of the message
    MLP is three accumulating bf16 matmuls into PSUM (src panel x W1s, dst
    panel x W1t, edge-attr block x [W1e; b1] packed 4-up with tile_position).
    ReLU to bf16, then aggregation by dst via a one-hot indicator matmul
    accumulated in PSUM ([f, n]).  The indicator is built on-chip per chunk:
    is_equal(iota_row, dslot_column), alternating DVE/GPSIMD.  W_msg2 is
    applied post-aggregation (segment_sum is linear) with deg x b2 as a
    rank-1 matmul.  The gated update MLP runs in bf16; LayerNorm stats
    (mean/var) are computed on DVE and the centered rows buffered in SBUF.
  * Pass 3: one Rsqrt over all tiles' variances (single sqrt-table load),
    then scale/affine per tile and store.  Only two activation-table loads
    total (sigmoid set + sqrt set); chunk ReLUs use the filler relu entry.
"""

import numpy as np
import ml_dtypes

P = 128
N_NODES = 50000
N_EDGES = 640000
H = 128
EDGE_DIM = 3
NC = 8
PC = N_NODES // NC            # 6250 nodes per core
NT = (PC + P - 1) // P        # 49 tiles per core
LAST_BASE = PC - P            # 6122: base of (overlapping) last tile

_kernel_cache = {}


# --------------------------------------------------------------------------
# Host-side preprocessing
# --------------------------------------------------------------------------

def _tile_base(t):
    return LAST_BASE if t == NT - 1 else t * P


def _prep(x, edge_index, edge_attr):
    """Bucket edges per (core, tile); build panels + metadata per core."""
    bf = ml_dtypes.bfloat16
    src = np.asarray(edge_index[0], dtype=np.int64)
    dst = np.asarray(edge_index[1], dtype=np.int64)
    ea = np.asarray(edge_attr, dtype=np.float32)
    x_bf = x.astype(bf)

    per_core = []
    k_ct = np.zeros((NC, NT), np.int64)
    for c in range(NC):
        m = (dst // PC) == c
        d_l = dst[m] - c * PC
        s_l = src[m]
        ea_l = ea[m]
        tiles = []
        for t in range(NT):
            base = _tile_base(t)
            lo, hi = t * P, min((t + 1) * P, PC)
            tm = (d_l >= lo) & (d_l < hi)
            ss, ds, eat = s_l[tm], d_l[tm] - base, ea_l[tm]
            o = np.argsort(ds, kind="stable")
            ss, ds, eat = ss[o], ds[o], eat[o]
            k_ct[c, t] = len(ss)
            tiles.append((ss, ds, eat))
        per_core.append(tiles)

    cpt = np.ceil(k_ct / P).astype(int).max(axis=0)     # chunks per tile
    cpt_max = int(cpt.max())
    g4 = (cpt_max + 3) // 4
    offs = np.concatenate([[0], np.cumsum(cpt * P)])
    S = int(offs[-1])

    maps = []
    for c in range(NC):
        srcp = np.zeros((P, S), bf)
        dstp = np.zeros((P, S), bf)
        ea4 = np.zeros((NT, P, g4 * P), bf)
        dcol = np.full((P, NT * cpt_max), -1.0, np.float32)
        deg = np.zeros((1, NT * P), bf)
        for t in range(NT):
            ss, ds, eat = per_core[c][t]
            k = len(ss)
            et = cpt[t] * P
            off = int(offs[t])
            base = _tile_base(t)
            srcp[:, off : off + k] = x_bf[ss].T
            dstp[:, off : off + k] = x_bf[c * PC + base + ds].T
            dc = np.full(et, -1.0, np.float32)
            dc[:k] = ds
            dcol[:, t * cpt_max : t * cpt_max + cpt[t]] = (
                dc.reshape(cpt[t], P).T
            )
            eafull = np.zeros((et, EDGE_DIM + 1), np.float32)
            eafull[:k, :EDGE_DIM] = eat
            eafull[:k, EDGE_DIM] = 1.0                   # ones row -> +b1
            for ci in range(cpt[t]):
                j = ci % 4
                ea4[t, 32 * j : 32 * j + EDGE_DIM + 1,
                    (ci // 4) * P : (ci // 4 + 1) * P] = (
                    eafull[ci * P : (ci + 1) * P].T.astype(bf)
                )
            degf = np.zeros(P, np.float32)
            np.add.at(degf, ds, 1.0)
            deg[0, t * P : (t + 1) * P] = degf.astype(bf)
        x_sl = np.zeros((P, NT * H), np.float32)
        xT_sl = np.zeros((P, NT * H), bf)
        for t in range(NT):
            b = _tile_base(t) + c * PC
            x_sl[:, t * H : (t + 1) * H] = x[b : b + P]
            xT_sl[:, t * H : (t + 1) * H] = x[b : b + P].T.astype(bf)
        iotar = np.tile(np.arange(P, dtype=np.float32)[None, :], (P, 1)).astype(bf)
        maps.append(dict(srcp=srcp, dstp=dstp, ea4=ea4, dcol=dcol,
                         deg=deg, x_sl=x_sl, xT_sl=xT_sl, iotar=iotar))
    return maps, tuple(int(v) for v in cpt), cpt_max, g4


def _weight_map(kw):
    bf = ml_dtypes.bfloat16
    w1e4 = np.zeros((P, H), np.float32)
    for j in range(4):
        w1e4[32 * j : 32 * j + EDGE_DIM] = kw["W_msg1"][2 * H :]
        w1e4[32 * j + EDGE_DIM] = kw["b_msg1"]
    return dict(
        w1s=kw["W_msg1"][:H].astype(bf),
        w1t=kw["W_msg1"][H : 2 * H].astype(bf),
        w1e4=w1e4.astype(bf),
        w2=kw["W_msg2"].astype(np.float32),
        b2r=kw["b_msg2"].astype(np.float32)[None, :],
        wgx=kw["W_gate"][:H].astype(bf),
        wga=kw["W_gate"][H:].astype(bf),
        bgr=kw["b_gate"].astype(bf)[None, :],
        wu1x=kw["W_upd1"][:H].astype(bf),
        wu1a=kw["W_upd1"][H:].astype(bf),
        bu1c=kw["b_upd1"].astype(np.float32)[:, None],
        wu2=kw["W_upd2"].astype(bf),
        bu2r=kw["b_upd2"].astype(bf)[None, :],
        gammab=np.tile(kw["ln_gamma"].astype(np.float32)[None, :], (P, 1)),
        betab=np.tile(kw["ln_beta"].astype(np.float32)[None, :], (P, 1)),
    )


# --------------------------------------------------------------------------
# Bass kernel builder
# --------------------------------------------------------------------------

def _build(cpt, cpt_max, g4):
    import concourse.bacc as bacc
    import concourse.tile as tile
    from concourse import mybir

    f32 = mybir.dt.float32
    bf16 = mybir.dt.bfloat16
    Alu = mybir.AluOpType
    Act = mybir.ActivationFunctionType
    Axis = mybir.AxisListType

    nc = bacc.Bacc("TRN2", target_bir_lowering=False, debug=False, num_devices=NC)

    offs = [0]
    for t in range(NT):
        offs.append(offs[-1] + cpt[t] * P)
    S = offs[-1]

    # ---- DRAM I/O ----
    srcp_d = nc.dram_tensor("srcp", [P, S], bf16, kind="ExternalInput")
    dstp_d = nc.dram_tensor("dstp", [P, S], bf16, kind="ExternalInput")
    ea4_d = nc.dram_tensor("ea4", [NT, P, g4 * P], bf16, kind="ExternalInput")
    dcol_d = nc.dram_tensor("dcol", [P, NT * cpt_max], f32, kind="ExternalInput")
    deg_d = nc.dram_tensor("deg", [1, NT * P], bf16, kind="ExternalInput")
    x_sl_d = nc.dram_tensor("x_sl", [P, NT * H], f32, kind="ExternalInput")
    xT_sl_d = nc.dram_tensor("xT_sl", [P, NT * H], bf16, kind="ExternalInput")
    iotar_d = nc.dram_tensor("iotar", [P, P], bf16, kind="ExternalInput")

    wnames = [
        ("w1s", [H, H], bf16), ("w1t", [H, H], bf16), ("w1e4", [P, H], bf16),
        ("w2", [H, H], f32), ("b2r", [1, H], f32),
        ("wgx", [H, H], bf16), ("wga", [H, H], bf16), ("bgr", [1, H], bf16),
        ("wu1x", [H, H], bf16), ("wu1a", [H, H], bf16), ("bu1c", [P, 1], f32),
        ("wu2", [H, H], bf16), ("bu2r", [1, H], bf16),
        ("gammab", [P, H], f32), ("betab", [P, H], f32),
    ]
    wd = {n: nc.dram_tensor(n, s, d, kind="ExternalInput") for n, s, d in wnames}

    out_d = nc.dram_tensor("out", [NT * P, H], f32, kind="ExternalOutput")

    with tile.TileContext(nc) as tc:
        with (
            tc.tile_pool(name="const", bufs=1) as cpool,
            tc.tile_pool(name="panel", bufs=3) as panel_pool,
            tc.tile_pool(name="meta", bufs=3) as meta_pool,
            tc.tile_pool(name="work", bufs=10) as work_pool,
            tc.tile_pool(name="indp", bufs=2 * cpt_max) as ind_pool,
            tc.tile_pool(name="upd", bufs=2) as upd_pool,
            tc.tile_pool(name="ps_h1", bufs=4, space="PSUM") as ps_h1,
            tc.tile_pool(name="ps_agg", bufs=2, space="PSUM") as ps_agg,
            tc.tile_pool(name="ps_upd", bufs=2, space="PSUM") as ps_upd,
        ):
            # ---- constants + whole-array preloads ----
            iota_r = cpool.tile([P, P], bf16)
            nc.sync.dma_start(out=iota_r[:], in_=iotar_d[:])
            ones_row = cpool.tile([1, P], bf16)
            nc.vector.memset(ones_row[:], 1.0)
            eps_col = cpool.tile([P, 1], f32)
            nc.vector.memset(eps_col[:], 1e-5)
            W = {}
            for n, s, d in wnames:
                W[n] = cpool.tile(s, d, tag=n, name=f"w_{n}")
                nc.sync.dma_start(out=W[n][:], in_=wd[n][:])
            cent_buf = cpool.tile([P, NT * P], f32, tag="cent")
            var_buf = cpool.tile([P, NT], f32, tag="var")
            rstd_buf = cpool.tile([P, NT], f32, tag="rstd")
            dcol_sb = cpool.tile([P, NT * cpt_max], f32, tag="dcol")
            nc.scalar.dma_start(out=dcol_sb[:], in_=dcol_d[:])
            deg_all = cpool.tile([1, NT * P], bf16, tag="dega")
            nc.scalar.dma_start(out=deg_all[:], in_=deg_d[:])
            xt_all = cpool.tile([P, NT * H], f32, tag="xta")
            nc.sync.dma_start(out=xt_all[:], in_=x_sl_d[:])
            xT_all = cpool.tile([P, NT * H], bf16, tag="xTa")
            nc.sync.dma_start(out=xT_all[:], in_=xT_sl_d[:])

            for t in range(NT):
                ct = cpt[t]
                et = ct * P
                g4t = (ct + 3) // 4
                off = offs[t]
                # ---- per-tile loads ----
                src_sb = panel_pool.tile([P, cpt_max * P], bf16, tag="src")
                nc.sync.dma_start(out=src_sb[:, :et], in_=srcp_d[:, off : off + et])
                dst_sb = panel_pool.tile([P, cpt_max * P], bf16, tag="dst")
                nc.sync.dma_start(out=dst_sb[:, :et], in_=dstp_d[:, off : off + et])
                ea_sb = meta_pool.tile([P, g4 * P], bf16, tag="ea")
                nc.scalar.dma_start(out=ea_sb[:, : g4t * P], in_=ea4_d[t, :, : g4t * P])

                # ---- per-chunk dst one-hot indicators (DVE, hoisted) ----
                inds = {}
                for ci in range(ct):
                    ind = ind_pool.tile([P, P], bf16, tag="ind",
                                        name=f"ind_{t}_{ci}")
                    inds[ci] = ind
                    nc.vector.tensor_scalar(
                        out=ind[:], in0=iota_r[:],
                        scalar1=dcol_sb[:, t * cpt_max + ci : t * cpt_max + ci + 1],
                        scalar2=None, op0=Alu.is_equal)

                # ---- chunk loop (groups of 8) ----
                aggA_ps = ps_agg.tile([P, P], f32, tag="agg")
                for c0 in range(0, ct, 8):
                    grp = list(range(c0, min(c0 + 8, ct)))
                    h1s = {}
                    for ci in grp:
                        cs = slice(ci * P, (ci + 1) * P)
                        h1 = ps_h1.tile([P, P], f32, tag="h1", name=f"h1_{t}_{ci}")
                        h1s[ci] = h1
                        nc.tensor.matmul(out=h1[:], lhsT=src_sb[:, cs],
                                         rhs=W["w1s"][:], start=True, stop=False)
                        nc.tensor.matmul(out=h1[:], lhsT=dst_sb[:, cs],
                                         rhs=W["w1t"][:], start=False, stop=False)
                    for ci in grp:
                        j = ci % 4
                        nc.tensor.matmul(
                            out=h1s[ci][:],
                            lhsT=ea_sb[32 * j : 32 * j + EDGE_DIM + 1,
                                       (ci // 4) * P : (ci // 4 + 1) * P],
                            rhs=W["w1e4"][32 * j : 32 * j + EDGE_DIM + 1, :],
                            start=False, stop=True, tile_position=(32 * j, 0),
                        )
                    for ci in grp:
                        A_sb = work_pool.tile([P, P], bf16, tag="A",
                                              name=f"A_{t}_{ci}")
                        if ci % 2 == 0:
                            nc.scalar.activation(out=A_sb[:], in_=h1s[ci][:],
                                                 func=Act.Relu)
                        else:
                            nc.vector.tensor_scalar(
                                out=A_sb[:], in0=h1s[ci][:], scalar1=0.0,
                                scalar2=None, op0=Alu.max)
                        nc.tensor.matmul(out=aggA_ps[:], lhsT=A_sb[:],
                                         rhs=inds[ci][:], start=(ci == 0),
                                         stop=(ci == ct - 1))

                # ---- agg2T = W2.T @ aggA + b2 (x) deg   ([g, n], fp32) ----
                aggA_sb = upd_pool.tile([P, P], f32, tag="aggA")
                nc.vector.tensor_copy(out=aggA_sb[:], in_=aggA_ps[:])
                deg_f32 = upd_pool.tile([1, P], f32, tag="degf")
                nc.vector.tensor_copy(out=deg_f32[:],
                                      in_=deg_all[0:1, t * P : (t + 1) * P])
                agg2_ps = ps_upd.tile([P, P], f32, tag="u")
                nc.tensor.matmul(out=agg2_ps[:], lhsT=W["w2"][:], rhs=aggA_sb[:],
                                 start=True, stop=False)
                nc.tensor.matmul(out=agg2_ps[:], lhsT=W["b2r"][:], rhs=deg_f32[:],
                                 start=False, stop=True)
                agg2T = upd_pool.tile([P, P], bf16, tag="agg2T")
                nc.vector.tensor_copy(out=agg2T[:], in_=agg2_ps[:])

                # ---- gate = sigmoid([x, agg] @ W_gate + b_gate) ----
                gate_ps = ps_upd.tile([P, P], f32, tag="u")
                nc.tensor.matmul(out=gate_ps[:], lhsT=xT_all[:, t * H : (t + 1) * H], rhs=W["wgx"][:],
                                 start=True, stop=False)
                nc.tensor.matmul(out=gate_ps[:], lhsT=agg2T[:], rhs=W["wga"][:],
                                 start=False, stop=False)
                nc.tensor.matmul(out=gate_ps[:], lhsT=ones_row[:], rhs=W["bgr"][:],
                                 start=False, stop=True)
                gate = upd_pool.tile([P, P], f32, tag="gate")
                nc.scalar.activation(out=gate[:], in_=gate_ps[:], func=Act.Sigmoid)

                # ---- update = relu([x, agg] @ W_upd1 + b_upd1) @ W_upd2 + b2
                u1_ps = ps_upd.tile([P, P], f32, tag="u")
                nc.tensor.matmul(out=u1_ps[:], lhsT=W["wu1x"][:], rhs=xT_all[:, t * H : (t + 1) * H],
                                 start=True, stop=False)
                nc.tensor.matmul(out=u1_ps[:], lhsT=W["wu1a"][:], rhs=agg2T[:],
                                 start=False, stop=True)
                u1b = upd_pool.tile([P, P], f32, tag="u1b")
                nc.vector.tensor_scalar(out=u1b[:], in0=u1_ps[:],
                                        scalar1=W["bu1c"][:, 0:1], scalar2=None,
                                        op0=Alu.add)
                UT = upd_pool.tile([P, P], bf16, tag="UT")
                nc.vector.tensor_scalar(out=UT[:], in0=u1b[:], scalar1=0.0,
                                        scalar2=None, op0=Alu.max)
                upd_ps = ps_upd.tile([P, P], f32, tag="u")
                nc.tensor.matmul(out=upd_ps[:], lhsT=UT[:], rhs=W["wu2"][:],
                                 start=True, stop=False)
                nc.tensor.matmul(out=upd_ps[:], lhsT=ones_row[:], rhs=W["bu2r"][:],
                                 start=False, stop=True)

                # ---- out0 = x + gate * (update - x); LN stats ----
                d1 = upd_pool.tile([P, P], f32, tag="d1")
                nc.vector.tensor_sub(out=d1[:], in0=upd_ps[:], in1=xt_all[:, t * H : (t + 1) * H])
                d2 = upd_pool.tile([P, P], f32, tag="d2")
                nc.vector.tensor_mul(out=d2[:], in0=d1[:], in1=gate[:])
                out0 = upd_pool.tile([P, P], f32, tag="out0")
                nc.vector.tensor_add(out=out0[:], in0=d2[:], in1=xt_all[:, t * H : (t + 1) * H])

                stat = upd_pool.tile([P, 2], f32, tag="stat")
                nc.vector.tensor_reduce(out=stat[:, 0:1], in_=out0[:],
                                        axis=Axis.X, op=Alu.add)
                nc.vector.tensor_scalar(out=stat[:, 1:2], in0=stat[:, 0:1],
                                        scalar1=1.0 / H, scalar2=None,
                                        op0=Alu.mult)
                cs_t = slice(t * P, (t + 1) * P)
                nc.vector.tensor_scalar(out=cent_buf[:, cs_t], in0=out0[:],
                                        scalar1=stat[:, 1:2], scalar2=None,
                                        op0=Alu.subtract)
                scr = upd_pool.tile([P, P], f32, tag="scr")
                nc.vector.tensor_mul(out=scr[:], in0=cent_buf[:, cs_t],
                                     in1=cent_buf[:, cs_t])
                nc.vector.tensor_reduce(out=var_buf[:, t : t + 1], in_=scr[:],
                                        axis=Axis.X, op=Alu.add)

            # ---- pass 3: rstd = 1/sqrt(var/H + eps); normalize; store ----
            sstd_buf = cpool.tile([P, NT], f32, tag="sstd")
            nc.scalar.activation(out=sstd_buf[:], in_=var_buf[:],
                                 func=Act.Sqrt, bias=eps_col[:, 0:1],
                                 scale=1.0 / H)
            nc.vector.reciprocal(out=rstd_buf[:], in_=sstd_buf[:])
            for t in range(NT):
                cs_t = slice(t * P, (t + 1) * P)
                nrm = upd_pool.tile([P, P], f32, tag="nrm")
                nc.vector.tensor_scalar(out=nrm[:], in0=cent_buf[:, cs_t],
                                        scalar1=rstd_buf[:, t : t + 1],
                                        scalar2=None, op0=Alu.mult)
                g1 = upd_pool.tile([P, P], f32, tag="g1")
                nc.vector.tensor_mul(out=g1[:], in0=nrm[:], in1=W["gammab"][:])
                outf = upd_pool.tile([P, P], f32, tag="outf")
                nc.vector.tensor_add(out=outf[:], in0=g1[:], in1=W["betab"][:])
                nc.sync.dma_start(out=out_d[t * P : (t + 1) * P, :], in_=outf[:])

    nc.compile()
    return nc


# --------------------------------------------------------------------------
# Public entry point
# --------------------------------------------------------------------------

def build_in_maps(**inputs):
    """Host prep: returns (nc-builder args, per-core input maps)."""
    x = np.asarray(inputs["x"], dtype=np.float32)
    maps, cpt, cpt_max, g4 = _prep(x, inputs["edge_index"], inputs["edge_attr"])
    wm = _weight_map(inputs)
    in_maps = []
    for c in range(NC):
        m = dict(maps[c])
        m.update(wm)
        in_maps.append(m)
    return (cpt, cpt_max, g4), in_maps


def get_kernel(build_args):
    if build_args not in _kernel_cache:
        _kernel_cache[build_args] = _build(*build_args)
    return _kernel_cache[build_args]


def assemble(results):
    """results: list of per-core out arrays [NT*P, H] -> [N_NODES, H]."""
    full = np.empty((N_NODES, H), np.float32)
    n_full = (NT - 1) * P          # 6144 rows from non-overlapping tiles
    off = n_full - LAST_BASE       # duplicated rows at start of last tile
    for c in range(NC):
        o = results[c]
        lo = c * PC
        full[lo : lo + n_full] = o[:n_full]
        full[lo + n_full : lo + PC] = o[n_full + off : n_full + off + (PC - n_full)]
    return full


def kernel(**inputs):
    import time
    from concourse.bass_utils import run_bass_kernel_spmd

    build_args, in_maps = build_in_maps(**inputs)
    nc = get_kernel(build_args)
    last_err = None
    for attempt in range(3):
        try:
            res = run_bass_kernel_spmd(nc, in_maps, list(range(NC)))
            outs = [res.results[c]["out"] for c in range(NC)]
            return assemble(outs)
        except Exception as e:  # transient device wedge: retry
            last_err = e
            time.sleep(2.0)
    raise last_err


if __name__ == "__main__":
    import reference

    inputs = {k: np.asarray(v) for k, v in reference.setup_inputs().items()}
    out = kernel(**inputs)
    exp = np.asarray(reference.reference(**reference.setup_inputs()))
    err = np.abs(out - exp).max() / (np.abs(exp).max() + 1e-12)
    print("Relative error:", err)


# revision 32
# speedup vs baseline: 1.0358x; 1.0358x over previous
"""Trainium2 Bass kernel for EnhancedMessageLayer (GNN message passing).

Strategy (8 NeuronCores, no collectives):
  * Nodes split into 8 contiguous slices of 6250; every edge is owned by the
    core that owns its dst node.  Each core computes the full layer for its
    node slice; host concatenates.  Per core, nodes are processed in 49 tiles
    of 128 (last tile overlaps; duplicated rows discarded on host).
  * Host prep lays out, per (core, tile), the transposed bf16 panels
    x[src].T and x[dst].T ([h, e] slot layout, matmul-stationary-ready),
    edge_attr blocks (with a ones-row so b_msg1 rides the same matmul), the
    per-slot dst-slot columns, and per-node degree.  No device-side gather,
    no dense one-hot uploads.
  * Device, pass 1+2 (per tile): per 128-edge chunk, layer 1 of the message
    MLP is three accumulating bf16 matmuls into PSUM (src panel x W1s, dst
    panel x W1t, edge-attr block x [W1e; b1] packed 4-up with tile_position).
    ReLU to bf16, then aggregation by dst via a one-hot indicator matmul
    accumulated in PSUM ([f, n]).  The indicator is built on-chip per chunk:
    is_equal(iota_row, dslot_column), alternating DVE/GPSIMD.  W_msg2 is
    applied post-aggregation (segment_sum is linear) with deg x b2 as a
    rank-1 matmul.  The gated update MLP runs in bf16; LayerNorm stats
    (mean/var) are computed on DVE and the centered rows buffered in SBUF.
  * Pass 3: one Rsqrt over all tiles' variances (single sqrt-table load),
    then scale/affine per tile and store.  Only two activation-table loads
    total (sigmoid set + sqrt set); chunk ReLUs use the filler relu entry.
"""

import numpy as np
import ml_dtypes

P = 128
N_NODES = 50000
N_EDGES = 640000
H = 128
EDGE_DIM = 3
NC = 8
PC = N_NODES // NC            # 6250 nodes per core
NT = (PC + P - 1) // P        # 49 tiles per core
LAST_BASE = PC - P            # 6122: base of (overlapping) last tile

_kernel_cache = {}


# --------------------------------------------------------------------------
# Host-side preprocessing
# --------------------------------------------------------------------------

def _tile_base(t):
    return LAST_BASE if t == NT - 1 else t * P


def _prep(x, edge_index, edge_attr):
    """Bucket edges per (core, tile); build panels + metadata per core."""
    bf = ml_dtypes.bfloat16
    src = np.asarray(edge_index[0], dtype=np.int64)
    dst = np.asarray(edge_index[1], dtype=np.int64)
    ea = np.asarray(edge_attr, dtype=np.float32)
    x_bf = x.astype(bf)

    per_core = []
    k_ct = np.zeros((NC, NT), np.int64)
    for c in range(NC):
        m = (dst // PC) == c
        d_l = dst[m] - c * PC
        s_l = src[m]
        ea_l = ea[m]
        tiles = []
        for t in range(NT):
            base = _tile_base(t)
            lo, hi = t * P, min((t + 1) * P, PC)
            tm = (d_l >= lo) & (d_l < hi)
            ss, ds, eat = s_l[tm], d_l[tm] - base, ea_l[tm]
            o = np.argsort(ds, kind="stable")
            ss, ds, eat = ss[o], ds[o], eat[o]
            k_ct[c, t] = len(ss)
            tiles.append((ss, ds, eat))
        per_core.append(tiles)

    cpt = np.ceil(k_ct / P).astype(int).max(axis=0)     # chunks per tile
    cpt_max = int(cpt.max())
    g4 = (cpt_max + 3) // 4
    offs = np.concatenate([[0], np.cumsum(cpt * P)])
    S = int(offs[-1])

    maps = []
    for c in range(NC):
        srcp = np.zeros((P, S), bf)
        dstp = np.zeros((P, S), bf)
        ea4 = np.zeros((NT, P, g4 * P), bf)
        dcol = np.full((P, NT * cpt_max), -1.0, np.float32)
        deg = np.zeros((1, NT * P), bf)
        for t in range(NT):
            ss, ds, eat = per_core[c][t]
            k = len(ss)
            et = cpt[t] * P
            off = int(offs[t])
            base = _tile_base(t)
            srcp[:, off : off + k] = x_bf[ss].T
            dstp[:, off : off + k] = x_bf[c * PC + base + ds].T
            dc = np.full(et, -1.0, np.float32)
            dc[:k] = ds
            dcol[:, t * cpt_max : t * cpt_max + cpt[t]] = (
                dc.reshape(cpt[t], P).T
            )
            eafull = np.zeros((et, EDGE_DIM + 1), np.float32)
            eafull[:k, :EDGE_DIM] = eat
            eafull[:k, EDGE_DIM] = 1.0                   # ones row -> +b1
            for ci in range(cpt[t]):
                j = ci % 4
                ea4[t, 32 * j : 32 * j + EDGE_DIM + 1,
                    (ci // 4) * P : (ci // 4 + 1) * P] = (
                    eafull[ci * P : (ci + 1) * P].T.astype(bf)
                )
            degf = np.zeros(P, np.float32)
            np.add.at(degf, ds, 1.0)
            deg[0, t * P : (t + 1) * P] = degf.astype(bf)
        x_sl = np.zeros((P, NT * H), np.float32)
        xT_sl = np.zeros((P, NT * H), bf)
        for t in range(NT):
            b = _tile_base(t) + c * PC
            x_sl[:, t * H : (t + 1) * H] = x[b : b + P]
            xT_sl[:, t * H : (t + 1) * H] = x[b : b + P].T.astype(bf)
        iotar = np.tile(np.arange(P, dtype=np.float32)[None, :], (P, 1))
        maps.append(dict(srcp=srcp, dstp=dstp, ea4=ea4, dcol=dcol,
                         deg=deg, x_sl=x_sl, xT_sl=xT_sl, iotar=iotar))
    return maps, tuple(int(v) for v in cpt), cpt_max, g4


def _weight_map(kw):
    bf = ml_dtypes.bfloat16
    w1e4 = np.zeros((P, H), np.float32)
    for j in range(4):
        w1e4[32 * j : 32 * j + EDGE_DIM] = kw["W_msg1"][2 * H :]
        w1e4[32 * j + EDGE_DIM] = kw["b_msg1"]
    return dict(
        w1s=kw["W_msg1"][:H].astype(bf),
        w1t=kw["W_msg1"][H : 2 * H].astype(bf),
        w1e4=w1e4.astype(bf),
        w2=kw["W_msg2"].astype(np.float32),
        b2r=kw["b_msg2"].astype(np.float32)[None, :],
        wgx=kw["W_gate"][:H].astype(bf),
        wga=kw["W_gate"][H:].astype(bf),
        bgr=kw["b_gate"].astype(bf)[None, :],
        wu1x=kw["W_upd1"][:H].astype(bf),
        wu1a=kw["W_upd1"][H:].astype(bf),
        bu1c=kw["b_upd1"].astype(np.float32)[:, None],
        wu2=kw["W_upd2"].astype(bf),
        bu2r=kw["b_upd2"].astype(bf)[None, :],
        gammab=np.tile(kw["ln_gamma"].astype(np.float32)[None, :], (P, 1)),
        betab=np.tile(kw["ln_beta"].astype(np.float32)[None, :], (P, 1)),
    )


# --------------------------------------------------------------------------
# Bass kernel builder
# --------------------------------------------------------------------------

def _build(cpt, cpt_max, g4):
    import concourse.bacc as bacc
    import concourse.tile as tile
    from concourse import mybir

    f32 = mybir.dt.float32
    bf16 = mybir.dt.bfloat16
    Alu = mybir.AluOpType
    Act = mybir.ActivationFunctionType
    Axis = mybir.AxisListType

    nc = bacc.Bacc("TRN2", target_bir_lowering=False, debug=False, num_devices=NC)

    offs = [0]
    for t in range(NT):
        offs.append(offs[-1] + cpt[t] * P)
    S = offs[-1]

    # ---- DRAM I/O ----
    srcp_d = nc.dram_tensor("srcp", [P, S], bf16, kind="ExternalInput")
    dstp_d = nc.dram_tensor("dstp", [P, S], bf16, kind="ExternalInput")
    ea4_d = nc.dram_tensor("ea4", [NT, P, g4 * P], bf16, kind="ExternalInput")
    dcol_d = nc.dram_tensor("dcol", [P, NT * cpt_max], f32, kind="ExternalInput")
    deg_d = nc.dram_tensor("deg", [1, NT * P], bf16, kind="ExternalInput")
    x_sl_d = nc.dram_tensor("x_sl", [P, NT * H], f32, kind="ExternalInput")
    xT_sl_d = nc.dram_tensor("xT_sl", [P, NT * H], bf16, kind="ExternalInput")
    iotar_d = nc.dram_tensor("iotar", [P, P], f32, kind="ExternalInput")

    wnames = [
        ("w1s", [H, H], bf16), ("w1t", [H, H], bf16), ("w1e4", [P, H], bf16),
        ("w2", [H, H], f32), ("b2r", [1, H], f32),
        ("wgx", [H, H], bf16), ("wga", [H, H], bf16), ("bgr", [1, H], bf16),
        ("wu1x", [H, H], bf16), ("wu1a", [H, H], bf16), ("bu1c", [P, 1], f32),
        ("wu2", [H, H], bf16), ("bu2r", [1, H], bf16),
        ("gammab", [P, H], f32), ("betab", [P, H], f32),
    ]
    wd = {n: nc.dram_tensor(n, s, d, kind="ExternalInput") for n, s, d in wnames}

    out_d = nc.dram_tensor("out", [NT * P, H], f32, kind="ExternalOutput")

    with tile.TileContext(nc) as tc:
        with (
            tc.tile_pool(name="const", bufs=1) as cpool,
            tc.tile_pool(name="panel", bufs=3) as panel_pool,
            tc.tile_pool(name="meta", bufs=3) as meta_pool,
            tc.tile_pool(name="work", bufs=10) as work_pool,
            tc.tile_pool(name="indp", bufs=2 * cpt_max) as ind_pool,
            tc.tile_pool(name="upd", bufs=2) as upd_pool,
            tc.tile_pool(name="ps_h1", bufs=3, space="PSUM") as ps_h1,
            tc.tile_pool(name="ps_agg", bufs=2, space="PSUM") as ps_agg,
            tc.tile_pool(name="ps_upd", bufs=2, space="PSUM") as ps_upd,
        ):
            # ---- constants + whole-array preloads ----
            iota_r = cpool.tile([P, P], f32)
            nc.sync.dma_start(out=iota_r[:], in_=iotar_d[:])
            ones_row = cpool.tile([1, P], bf16)
            nc.vector.memset(ones_row[:], 1.0)
            eps_col = cpool.tile([P, 1], f32)
            nc.vector.memset(eps_col[:], 1e-5)
            W = {}
            for n, s, d in wnames:
                W[n] = cpool.tile(s, d, tag=n, name=f"w_{n}")
                nc.sync.dma_start(out=W[n][:], in_=wd[n][:])
            cent_buf = cpool.tile([P, NT * P], f32, tag="cent")
            var_buf = cpool.tile([P, NT], f32, tag="var")
            rstd_buf = cpool.tile([P, NT], f32, tag="rstd")
            dcol_sb = cpool.tile([P, NT * cpt_max], f32, tag="dcol")
            nc.scalar.dma_start(out=dcol_sb[:], in_=dcol_d[:])
            deg_all = cpool.tile([1, NT * P], bf16, tag="dega")
            nc.scalar.dma_start(out=deg_all[:], in_=deg_d[:])
            xt_all = cpool.tile([P, NT * H], f32, tag="xta")
            xT_all = cpool.tile([P, NT * H], bf16, tag="xTa")

            for t in range(NT):
                ct = cpt[t]
                et = ct * P
                g4t = (ct + 3) // 4
                off = offs[t]
                # ---- per-tile loads ----
                src_sb = panel_pool.tile([P, cpt_max * P], bf16, tag="src")
                nc.sync.dma_start(out=src_sb[:, :et], in_=srcp_d[:, off : off + et])
                dst_sb = panel_pool.tile([P, cpt_max * P], bf16, tag="dst")
                nc.sync.dma_start(out=dst_sb[:, :et], in_=dstp_d[:, off : off + et])
                ea_sb = meta_pool.tile([P, g4 * P], bf16, tag="ea")
                nc.scalar.dma_start(out=ea_sb[:, : g4t * P], in_=ea4_d[t, :, : g4t * P])
                if t == 0:
                    nc.scalar.dma_start(out=xt_all[:], in_=x_sl_d[:])
                    nc.scalar.dma_start(out=xT_all[:], in_=xT_sl_d[:])

                # ---- per-chunk dst one-hot indicators (DVE, hoisted) ----
                inds = {}
                for ci in range(ct):
                    ind = ind_pool.tile([P, P], bf16, tag="ind",
                                        name=f"ind_{t}_{ci}")
                    inds[ci] = ind
                    nc.vector.tensor_scalar(
                        out=ind[:], in0=iota_r[:],
                        scalar1=dcol_sb[:, t * cpt_max + ci : t * cpt_max + ci + 1],
                        scalar2=None, op0=Alu.is_equal)

                # ---- chunk loop, software-pipelined (lag-2 relu, lag-4 agg)
                aggA_ps = ps_agg.tile([P, P], f32, tag="agg")
                h1s = {}
                A_sbs = {}
                LAG_R, LAG_A = 2, 4
                h1bank = None
                for ci in range(ct + LAG_A):
                    if ci < ct:
                        cs = slice(ci * P, (ci + 1) * P)
                        j = ci % 4
                        if j == 0:
                            h1bank = ps_h1.tile([P, 4 * P], f32, tag="h1",
                                                name=f"h1b_{t}_{ci // 4}")
                        h1 = h1bank[:, j * P : (j + 1) * P]
                        h1s[ci] = h1
                        nc.tensor.matmul(out=h1, lhsT=src_sb[:, cs],
                                         rhs=W["w1s"][:], start=True, stop=False)
                        nc.tensor.matmul(out=h1, lhsT=dst_sb[:, cs],
                                         rhs=W["w1t"][:], start=False, stop=False)
                        nc.tensor.matmul(
                            out=h1,
                            lhsT=ea_sb[32 * j : 32 * j + EDGE_DIM + 1,
                                       (ci // 4) * P : (ci // 4 + 1) * P],
                            rhs=W["w1e4"][32 * j : 32 * j + EDGE_DIM + 1, :],
                            start=False, stop=True, tile_position=(32 * j, 0),
                        )
                    cr = ci - LAG_R
                    if 0 <= cr < ct:
                        A_sb = work_pool.tile([P, P], bf16, tag="A",
                                              name=f"A_{t}_{cr}")
                        A_sbs[cr] = A_sb
                        if cr % 3 == 2:
                            nc.vector.tensor_scalar(
                                out=A_sb[:], in0=h1s[cr], scalar1=0.0,
                                scalar2=None, op0=Alu.max)
                        else:
                            nc.scalar.activation(out=A_sb[:], in_=h1s[cr],
                                                 func=Act.Relu)
                    ca_ = ci - LAG_A
                    if ca_ >= 0:
                        nc.tensor.matmul(out=aggA_ps[:], lhsT=A_sbs[ca_][:],
                                         rhs=inds[ca_][:], start=(ca_ == 0),
                                         stop=(ca_ == ct - 1))

                # ---- agg2T = W2.T @ aggA + b2 (x) deg   ([g, n], fp32) ----
                aggA_sb = upd_pool.tile([P, P], f32, tag="aggA")
                nc.vector.tensor_copy(out=aggA_sb[:], in_=aggA_ps[:])
                deg_f32 = upd_pool.tile([1, P], f32, tag="degf")
                nc.vector.tensor_copy(out=deg_f32[:],
                                      in_=deg_all[0:1, t * P : (t + 1) * P])
                agg2_ps = ps_upd.tile([P, P], f32, tag="u")
                nc.tensor.matmul(out=agg2_ps[:], lhsT=W["w2"][:], rhs=aggA_sb[:],
                                 start=True, stop=False)
                nc.tensor.matmul(out=agg2_ps[:], lhsT=W["b2r"][:], rhs=deg_f32[:],
                                 start=False, stop=True)
                agg2T = upd_pool.tile([P, P], bf16, tag="agg2T")
                nc.vector.tensor_copy(out=agg2T[:], in_=agg2_ps[:])

                # ---- gate = sigmoid([x, agg] @ W_gate + b_gate) ----
                gate_ps = ps_upd.tile([P, P], f32, tag="u")
                nc.tensor.matmul(out=gate_ps[:], lhsT=xT_all[:, t * H : (t + 1) * H], rhs=W["wgx"][:],
                                 start=True, stop=False)
                nc.tensor.matmul(out=gate_ps[:], lhsT=agg2T[:], rhs=W["wga"][:],
                                 start=False, stop=False)
                nc.tensor.matmul(out=gate_ps[:], lhsT=ones_row[:], rhs=W["bgr"][:],
                                 start=False, stop=True)
                gate = upd_pool.tile([P, P], f32, tag="gate")
                nc.scalar.activation(out=gate[:], in_=gate_ps[:], func=Act.Sigmoid)

                # ---- update = relu([x, agg] @ W_upd1 + b_upd1) @ W_upd2 + b2
                u1_ps = ps_upd.tile([P, P], f32, tag="u")
                nc.tensor.matmul(out=u1_ps[:], lhsT=W["wu1x"][:], rhs=xT_all[:, t * H : (t + 1) * H],
                                 start=True, stop=False)
                nc.tensor.matmul(out=u1_ps[:], lhsT=W["wu1a"][:], rhs=agg2T[:],
                                 start=False, stop=True)
                u1b = upd_pool.tile([P, P], f32, tag="u1b")
                nc.vector.tensor_scalar(out=u1b[:], in0=u1_ps[:],
                                        scalar1=W["bu1c"][:, 0:1], scalar2=None,
                                        op0=Alu.add)
                UT = upd_pool.tile([P, P], bf16, tag="UT")
                nc.vector.tensor_scalar(out=UT[:], in0=u1b[:], scalar1=0.0,
                                        scalar2=None, op0=Alu.max)
                upd_ps = ps_upd.tile([P, P], f32, tag="u")
                nc.tensor.matmul(out=upd_ps[:], lhsT=UT[:], rhs=W["wu2"][:],
                                 start=True, stop=False)
                nc.tensor.matmul(out=upd_ps[:], lhsT=ones_row[:], rhs=W["bu2r"][:],
                                 start=False, stop=True)

                # ---- out0 = x + gate * (update - x); LN stats ----
                d1 = upd_pool.tile([P, P], f32, tag="d1")
                nc.vector.tensor_sub(out=d1[:], in0=upd_ps[:], in1=xt_all[:, t * H : (t + 1) * H])
                d2 = upd_pool.tile([P, P], f32, tag="d2")
                nc.vector.tensor_mul(out=d2[:], in0=d1[:], in1=gate[:])
                out0 = upd_pool.tile([P, P], f32, tag="out0")
                nc.vector.tensor_add(out=out0[:], in0=d2[:], in1=xt_all[:, t * H : (t + 1) * H])

                stat = upd_pool.tile([P, 2], f32, tag="stat")
                nc.vector.tensor_reduce(out=stat[:, 0:1], in_=out0[:],
                                        axis=Axis.X, op=Alu.add)
                nc.vector.tensor_scalar(out=stat[:, 1:2], in0=stat[:, 0:1],
                                        scalar1=1.0 / H, scalar2=None,
                                        op0=Alu.mult)
                cs_t = slice(t * P, (t + 1) * P)
                nc.vector.tensor_scalar(out=cent_buf[:, cs_t], in0=out0[:],
                                        scalar1=stat[:, 1:2], scalar2=None,
                                        op0=Alu.subtract)
                scr = upd_pool.tile([P, P], f32, tag="scr")
                nc.vector.tensor_mul(out=scr[:], in0=cent_buf[:, cs_t],
                                     in1=cent_buf[:, cs_t])
                nc.vector.tensor_reduce(out=var_buf[:, t : t + 1], in_=scr[:],
                                        axis=Axis.X, op=Alu.add)

            # ---- pass 3: rstd = 1/sqrt(var/H + eps); normalize; store ----
            sstd_buf = cpool.tile([P, NT], f32, tag="sstd")
            nc.scalar.activation(out=sstd_buf[:], in_=var_buf[:],
                                 func=Act.Sqrt, bias=eps_col[:, 0:1],
                                 scale=1.0 / H)
            nc.vector.reciprocal(out=rstd_buf[:], in_=sstd_buf[:])
            for t in range(NT):
                cs_t = slice(t * P, (t + 1) * P)
                nrm = upd_pool.tile([P, P], f32, tag="nrm")
                nc.vector.tensor_scalar(out=nrm[:], in0=cent_buf[:, cs_t],
                                        scalar1=rstd_buf[:, t : t + 1],
                                        scalar2=None, op0=Alu.mult)
                g1 = upd_pool.tile([P, P], f32, tag="g1")
                nc.vector.tensor_mul(out=g1[:], in0=nrm[:], in1=W["gammab"][:])
                outf = upd_pool.tile([P, P], f32, tag="outf")
                nc.vector.tensor_add(out=outf[:], in0=g1[:], in1=W["betab"][:])
                nc.sync.dma_start(out=out_d[t * P : (t + 1) * P, :], in_=outf[:])

    nc.compile()
    return nc


# --------------------------------------------------------------------------
# Public entry point
# --------------------------------------------------------------------------

def build_in_maps(**inputs):
    """Host prep: returns (nc-builder args, per-core input maps)."""
    x = np.asarray(inputs["x"], dtype=np.float32)
    maps, cpt, cpt_max, g4 = _prep(x, inputs["edge_index"], inputs["edge_attr"])
    wm = _weight_map(inputs)
    in_maps = []
    for c in range(NC):
        m = dict(maps[c])
        m.update(wm)
        in_maps.append(m)
    return (cpt, cpt_max, g4), in_maps


def get_kernel(build_args):
    if build_args not in _kernel_cache:
        _kernel_cache[build_args] = _build(*build_args)
    return _kernel_cache[build_args]


def assemble(results):
    """results: list of per-core out arrays [NT*P, H] -> [N_NODES, H]."""
    full = np.empty((N_NODES, H), np.float32)
    n_full = (NT - 1) * P          # 6144 rows from non-overlapping tiles
    off = n_full - LAST_BASE       # duplicated rows at start of last tile
    for c in range(NC):
        o = results[c]
        lo = c * PC
        full[lo : lo + n_full] = o[:n_full]
        full[lo + n_full : lo + PC] = o[n_full + off : n_full + off + (PC - n_full)]
    return full


def kernel(**inputs):
    import time
    from concourse.bass_utils import run_bass_kernel_spmd

    build_args, in_maps = build_in_maps(**inputs)
    nc = get_kernel(build_args)
    last_err = None
    for attempt in range(3):
        try:
            res = run_bass_kernel_spmd(nc, in_maps, list(range(NC)))
            outs = [res.results[c]["out"] for c in range(NC)]
            return assemble(outs)
        except Exception as e:  # transient device wedge: retry
            last_err = e
            time.sleep(2.0)
    raise last_err


if __name__ == "__main__":
    import reference

    inputs = {k: np.asarray(v) for k, v in reference.setup_inputs().items()}
    out = kernel(**inputs)
    exp = np.asarray(reference.reference(**reference.setup_inputs()))
    err = np.abs(out - exp).max() / (np.abs(exp).max() + 1e-12)
    print("Relative error:", err)
